# revision 7
# baseline (speedup 1.0000x reference)
"""DeepSets APE encoder kernel for Trainium2 (8 NeuronCores, SPMD).

Computation (per reference):
  t = relu(ape @ W1 + b1) @ W2 + b2                      # [6016, 32] per-node
  out[pair(b,i,j)] = t[node(b,i)] + t[node(b,j)]         # [2349056, 32]

Device strategy:
  - Every core computes the tiny MLP for all nodes (redundant, cheap).
  - v = t (with b2 folded as v = h@W2 + b2); rpe = v_i + v_j.
  - The pairwise broadcast-add runs on the Tensor engine as one K=66
    float32r matmul per [i-chunk, 512-col] tile:
      lhsT rows 0-31  = vT_hi (f32r high part of v, transposed)
           rows 32-63 = vT_lo (residual, f32r)
           rows 64-65 = ones
      rhs  rows 0-31  = tiled identity E (delta[k',k] pattern)
           rows 32-63 = E again
           row  64    = v_hi flattened row for the graph's nodes
           row  65    = v_lo flattened row
    giving psum[i, j*32+k] = v[i,k] + v[j,k] at full fp32 precision.
  - PSUM -> SBUF staging copies alternate between Scalar and Vector engines,
    staging strips DMA to DRAM.
  - Work is sharded across the 8 cores at i-chunk granularity (static greedy
    bin-packing, ~1.3% imbalance); each core executes its own If(pid==c)
    branch and writes its own output buffer, reassembled on host.
"""

import sys

if "/opt/trn_rl_repo" not in sys.path:
    sys.path.insert(0, "/opt/trn_rl_repo")

import numpy as np

# ---------------- problem constants (hardcoded) ----------------
B = 16
IN_DIM, HID, OUT = 16, 64, 32
SIZES = [256 + 16 * g for g in range(B)]
N_TOT = sum(SIZES)  # 6016
STARTS = [0] * B
for g in range(1, B):
    STARTS[g] = STARTS[g - 1] + SIZES[g - 1]
PAIR_START = [0] * B
for g in range(1, B):
    PAIR_START[g] = PAIR_START[g - 1] + SIZES[g - 1] ** 2
N_PAIRS = sum(n * n for n in SIZES)  # 2349056
NCORES = 8
NCHUNKS = N_TOT // 128  # 47 (exact)
W_E = max(SIZES) * OUT  # 15872
JC = 512  # matmul moving-dim chunk (one PSUM bank of f32)
STRIP = 2048  # staging strip width in floats (4 matmul chunks)


def _make_assignment():
    """Greedy bin-pack i-chunks (<=128 rows of one graph) across 8 cores.

    Returns per-core ordered work lists: [(g, i0, ilen), ...] grouped by
    graph, plus per-core total output rows.
    """
    chunks = []
    for g, n in enumerate(SIZES):
        i0 = 0
        while i0 < n:
            ilen = min(128, n - i0)
            chunks.append((g, i0, ilen, ilen * n))
            i0 += ilen
    chunks.sort(key=lambda c: -c[3])
    loads = [0] * NCORES
    assign = [[] for _ in range(NCORES)]
    for c in chunks:
        k = min(range(NCORES), key=lambda i: loads[i])
        loads[k] += c[3]
        assign[k].append(c[:3])
    # group by graph, keep deterministic order
    for k in range(NCORES):
        assign[k].sort(key=lambda c: (c[0], c[1]))
    rows = [sum(il * SIZES[g] for g, _, il in a) for a in assign]
    return assign, rows


ASSIGN, CORE_ROWS = _make_assignment()
R_MAX = max(CORE_ROWS)

_compiled = None


def _build_program():
    import concourse.bacc as bacc
    import concourse.mybir as mybir
    import concourse.tile as tile
    from concourse.bass import AP

    f32 = mybir.dt.float32
    f32r = mybir.dt.float32r
    ACT_COPY = mybir.ActivationFunctionType.Copy
    ACT_RELU = mybir.ActivationFunctionType.Relu

    nc = bacc.Bacc("TRN2", target_bir_lowering=False, debug=False, num_devices=NCORES)

    apeT_d = nc.declare_dram_parameter("ape_t", [IN_DIM, N_TOT], f32, isOutput=False)
    W1_d = nc.declare_dram_parameter("W1", [IN_DIM, HID], f32, isOutput=False)
    W2_d = nc.declare_dram_parameter("W2", [HID, OUT], f32, isOutput=False)
    b1_d = nc.declare_dram_parameter("b1c", [HID, 1], f32, isOutput=False)
    b2_d = nc.declare_dram_parameter("b2r", [1, OUT], f32, isOutput=False)
    E2_d = nc.declare_dram_parameter("E2", [64, W_E], f32, isOutput=False)
    id_d = nc.declare_dram_parameter("ident", [128, 128], f32, isOutput=False)
    out_d = nc.declare_dram_parameter("out", [R_MAX * OUT], f32, isOutput=True)

    vhi_d = nc.dram_tensor("vhi_flat", [N_TOT * OUT], f32r)
    vlo_d = nc.dram_tensor("vlo_flat", [N_TOT * OUT], f32r)

    with tile.TileContext(nc) as tc:
        with tc.tile_pool(name="persist", bufs=1) as pers:
            tTo = pers.tile([66, N_TOT], f32r)
            rhs = pers.tile([66, W_E], f32r)
            ident = pers.tile([128, 128], f32r)
            b2_sb = pers.tile([1, OUT], f32)
            ones1 = pers.tile([1, 128], f32)

            # ---------------- stage 0: constants + MLP + v split ----------------
            with (
                tc.tile_pool(name="s0", bufs=1) as s0,
                tc.tile_pool(name="s0w", bufs=3) as s0w,
                tc.tile_pool(name="s0p", bufs=2, space="PSUM") as s0p,
            ):
                apeT_sb = s0.tile([IN_DIM, N_TOT], f32)
                nc.sync.dma_start(apeT_sb[:, :], apeT_d[:, :])
                W1_sb = s0.tile([IN_DIM, HID], f32)
                nc.sync.dma_start(W1_sb[:, :], W1_d[:, :])
                W2_sb = s0.tile([HID, OUT], f32)
                nc.sync.dma_start(W2_sb[:, :], W2_d[:, :])
                b1_sb = s0.tile([HID, 1], f32)
                nc.sync.dma_start(b1_sb[:, :], b1_d[:, :])
                nc.sync.dma_start(b2_sb[:, :], b2_d[:, :])
                nc.sync.dma_start(ident[:, :], id_d[:, :].bitcast(f32r))
                nc.sync.dma_start(rhs[0:64, :], E2_d[:, :].bitcast(f32r))
                nc.vector.memset(ones1[:, :], 1.0)
                ones2 = s0.tile([2, N_TOT], f32)
                nc.vector.memset(ones2[:, :], 1.0)
                nc.vector.tensor_copy(tTo[64:66, :], ones2[:, :])

                # h1t[h, n] = relu((ape @ W1).T + b1): lhsT=W1 [16,64], rhs=apeT
                h1t = s0.tile([HID, N_TOT], f32)
                q0 = 0
                while q0 < N_TOT:
                    w = min(JC, N_TOT - q0)
                    ph = s0p.tile([HID, JC], f32, tag="ph")
                    nc.tensor.matmul(
                        ph[:, :w],
                        lhsT=W1_sb[:, :],
                        rhs=apeT_sb[:, q0 : q0 + w],
                        start=True,
                        stop=True,
                    )
                    nc.scalar.activation(
                        h1t[:, q0 : q0 + w], ph[:, :w], ACT_RELU, bias=b1_sb[:, :]
                    )
                    q0 += w

                # per 128-node chunk: v = h.T@W2 + b2 -> split hi/lo f32r,
                # flat rows to DRAM, transposed halves into tTo
                for c in range(NCHUNKS):
                    pv = s0p.tile([128, OUT], f32, tag="pv")
                    nc.tensor.matmul(
                        pv[:, :],
                        lhsT=h1t[:, c * 128 : (c + 1) * 128],
                        rhs=W2_sb[:, :],
                        start=True,
                        stop=False,
                    )
                    nc.tensor.matmul(
                        pv[:, :],
                        lhsT=ones1[:, :],
                        rhs=b2_sb[:, :],
                        start=False,
                        stop=True,
                    )
                    vhi = s0w.tile([128, OUT], f32r, tag="vhi")
                    nc.vector.tensor_copy(vhi[:, :], pv[:, :])
                    vlo_f = s0w.tile([128, OUT], f32, tag="vlof")
                    nc.vector.tensor_sub(vlo_f[:, :], pv[:, :], vhi[:, :].bitcast(f32))
                    vlo = s0w.tile([128, OUT], f32r, tag="vlo")
                    nc.vector.tensor_copy(vlo[:, :], vlo_f[:, :])
                    nc.sync.dma_start(
                        vhi_d[c * 4096 : (c + 1) * 4096], vhi[:, :]
                    )
                    nc.sync.dma_start(
                        vlo_d[c * 4096 : (c + 1) * 4096], vlo[:, :]
                    )
                    pt = s0p.tile([32, 128], f32, tag="pt")
                    nc.tensor.transpose(
                        pt[:, :].bitcast(f32r), vhi[:, :], ident[:, :]
                    )
                    nc.vector.tensor_copy(tTo[0:32, c * 128 : (c + 1) * 128], pt[:, :])
                    pt2 = s0p.tile([32, 128], f32, tag="pt")
                    nc.tensor.transpose(
                        pt2[:, :].bitcast(f32r), vlo[:, :], ident[:, :]
                    )
                    nc.vector.tensor_copy(
                        tTo[32:64, c * 128 : (c + 1) * 128], pt2[:, :]
                    )

            # ---------------- stage 1: per-core pairwise tiles ----------------
            pid = nc.partition_id()
            with (
                tc.tile_pool(name="stage", bufs=3) as stp,
                tc.tile_pool(name="pp", bufs=8, space="PSUM") as pp,
            ):
                for core in range(NCORES):
                    with tc.If(pid == core):
                        copy_alt = 0
                        row_ofs = 0
                        cur_g = -1
                        for g, i0, ilen in ASSIGN[core]:
                            n = SIZES[g]
                            s = STARTS[g]
                            w32 = n * OUT
                            if g != cur_g:
                                nc.sync.dma_start(
                                    rhs[64:65, 0:w32],
                                    vhi_d[s * OUT : s * OUT + w32],
                                )
                                nc.sync.dma_start(
                                    rhs[65:66, 0:w32],
                                    vlo_d[s * OUT : s * OUT + w32],
                                )
                                cur_g = g
                            w0 = 0
                            while w0 < w32:
                                wlen = min(STRIP, w32 - w0)
                                st = stp.tile([128, STRIP], f32, tag="st")
                                jq = 0
                                while jq < wlen:
                                    ps = pp.tile([128, JC], f32, tag="ps")
                                    nc.tensor.matmul(
                                        ps[:ilen, :],
                                        lhsT=tTo[:, s + i0 : s + i0 + ilen],
                                        rhs=rhs[:, w0 + jq : w0 + jq + JC],
                                        start=True,
                                        stop=True,
                                    )
                                    if copy_alt % 2 == 0:
                                        nc.scalar.activation(
                                            st[:ilen, jq : jq + JC],
                                            ps[:ilen, :],
                                            ACT_COPY,
                                        )
                                    else:
                                        nc.vector.tensor_copy(
                                            st[:ilen, jq : jq + JC], ps[:ilen, :]
                                        )
                                    copy_alt += 1
                                    jq += JC
                                dst = AP(
                                    out_d,
                                    row_ofs * OUT + w0,
                                    [[w32, ilen], [1, wlen]],
                                )
                                nc.sync.dma_start(dst, st[:ilen, 0:wlen])
                                w0 += wlen
                            row_ofs += ilen * n
    nc.compile()
    return nc


def _get_compiled():
    global _compiled
    if _compiled is None:
        _compiled = _build_program()
    return _compiled


def _make_in_map(ape, W1, b1, W2, b2):
    ape = np.ascontiguousarray(np.asarray(ape, dtype=np.float32))
    E2 = np.tile(np.eye(OUT, dtype=np.float32), (2, W_E // OUT))
    return {
        "ape_t": np.ascontiguousarray(ape.T),
        "W1": np.ascontiguousarray(np.asarray(W1, dtype=np.float32)),
        "W2": np.ascontiguousarray(np.asarray(W2, dtype=np.float32)),
        "b1c": np.ascontiguousarray(
            np.asarray(b1, dtype=np.float32).reshape(HID, 1)
        ),
        "b2r": np.ascontiguousarray(
            np.asarray(b2, dtype=np.float32).reshape(1, OUT)
        ),
        "E2": E2,
        "ident": np.eye(128, dtype=np.float32),
    }


def _reassemble(results):
    out = np.empty((N_PAIRS, OUT), dtype=np.float32)
    for core in range(NCORES):
        arr = np.asarray(results[core]["out"]).reshape(-1)
        row_ofs = 0
        for g, i0, ilen in ASSIGN[core]:
            n = SIZES[g]
            rows = ilen * n
            src = arr[row_ofs * OUT : (row_ofs + rows) * OUT].reshape(rows, OUT)
            dst0 = PAIR_START[g] + i0 * n
            out[dst0 : dst0 + rows] = src
            row_ofs += rows
    return out


def _execute(in_map, trace=False, **trace_kwargs):
    from concourse.bass_utils import run_bass_kernel_spmd

    nc = _get_compiled()
    res = run_bass_kernel_spmd(
        nc,
        [dict(in_map) for _ in range(NCORES)],
        list(range(NCORES)),
        trace=trace,
        **trace_kwargs,
    )
    return _reassemble(res.results), res.exec_time_ns


def kernel(ape, W1, b1, W2, b2, batch=None, num_graphs=None, n_max=None, n_pairs=None):
    in_map = _make_in_map(ape, W1, b1, W2, b2)
    out, _ = _execute(in_map, trace=False)
    return out
